# revision 3
# baseline (speedup 1.0000x reference)
"""Entropic Sinkhorn loss kernel v2 for Trainium2 (8 NeuronCores, SPMD).

Math: loss = (sinkhorn(1 - img@txt.T) + sinkhorn((1 - img@txt.T).T)) / 2,
K = exp(100*S - 100), 5 iterations, P = u*K*v, loss = -mean(log_softmax(P)[i,i]).

Key ideas vs the baseline:
  * Row-scaled matrix A = diag(1/k_ii) K. Sinkhorn is exactly covariant under
    diagonal scaling: carrying x_t = kd*u_t, y_t = v_t (chain1) and
    w_t = u2_t, vt2_t = kd*v2_t (chain2) reproduces the reference exactly,
    with P recovered as diag products of the carried quantities (scales
    cancel). A has unit diagonal -> fits fp8 (off-diag underflows to 0 as it
    already does in bf16).
  * A stored fp8 in BOTH orientations (8MB + 8MB SBUF per core): kt8 (c-major,
    for row matvecs) and krm8 (r-major, for col matvecs). Every Sinkhorn
    stage is a dense 512-pair LDW+MM sweep on PE (mixed fp8 weights x bf16
    vectors, FWL) -- no transposes, copies or DVE matvecs in the pass loop.
  * Build: one GEMM (row-major S), ACT exp with per-partition bias
    (-100*S_rr) writes krm8 directly in fp8 and accumulates exact f32 A-row
    sums (chain2 seed). kt8 = PE transposes of krm8 (bitwise consistent).
    Chain1 seed from a seed sweep A8^T @ kd.
  * Stage-B AllGathers split in 2 halves and overlapped with compute; the
    next pass's stage A starts on the first half. Final loss reduced on
    device to [128, 2] per core and summed on host (no final collective).
"""

import numpy as np
import ml_dtypes

import concourse.bacc as bacc
import concourse.tile as tile
import concourse.mybir as mybir
from concourse import bass_utils
from concourse.masks import make_identity

B = 8192
D = 256
P = 128
NCORES = 8
R = B // NCORES            # 1024 rows per core
RT = R // P                # 8 row tiles per core
CT = B // P                # 64 col tiles
NCH = 16                   # GEMM free chunks per rb (8192 / 512)
CH = 512
N_ITER = 5
INV_REG = 100.0
BVAL = 1.0 / B

BF16 = mybir.dt.bfloat16
F32 = mybir.dt.float32
FP8 = mybir.dt.float8e4
Exp = mybir.ActivationFunctionType.Exp
Log = mybir.ActivationFunctionType.Ln
ADD = mybir.AluOpType.add
MULT = mybir.AluOpType.mult
BYPASS = mybir.AluOpType.bypass


def _build_bass(debug_taps=False, probe_rhs_const=False):
    nc = bacc.Bacc("TRN2", target_bir_lowering=False, debug=False,
                   num_devices=NCORES)
    dbg = {}
    if debug_taps:
        dbg["kd"] = nc.dram_tensor("dbg_kd", [P, RT], F32,
                                   kind="ExternalOutput")
        dbg["vt2seed"] = nc.dram_tensor("dbg_vt2seed", [P, RT], F32,
                                        kind="ExternalOutput")
        dbg["y1"] = nc.dram_tensor("dbg_y1", [P, CT], F32,
                                   kind="ExternalOutput")
        dbg["gg0"] = nc.dram_tensor("dbg_gg0", [P, CT], F32,
                                    kind="ExternalOutput")
        dbg["krm8"] = nc.dram_tensor("dbg_krm8", [P, RT, B], FP8,
                                     kind="ExternalOutput")
        dbg["kt8"] = nc.dram_tensor("dbg_kt8", [P, CT, R], FP8,
                                    kind="ExternalOutput")
        for t in range(1, N_ITER + 1):
            dbg[f"rr{t}"] = nc.dram_tensor(f"dbg_rr{t}", [P, RT, 2], F32,
                                           kind="ExternalOutput")
            dbg[f"gg{t}"] = nc.dram_tensor(f"dbg_gg{t}", [P, 2, CT], F32,
                                           kind="ExternalOutput")

    imgT = nc.dram_tensor("imgT", [P, 2, R], BF16, kind="ExternalInput")
    txtT = nc.dram_tensor("txtT", [P, 2, B], BF16, kind="ExternalInput")
    sd100_in = nc.dram_tensor("sd100", [P, RT], F32, kind="ExternalInput")
    dsel_in = nc.dram_tensor("dsel", [P, RT, CT], F32, kind="ExternalInput")
    out_loss = nc.dram_tensor("out_loss", [P, 2], F32, kind="ExternalOutput")

    RG = [list(range(NCORES))]
    # AG staging: t=0 seed [P, CT]; passes t=1..5: 2 halves of [P, CT] each
    ar_in = {}
    ar_out = {}
    ar_in[(0, 0)] = nc.dram_tensor("ar_in0", [P, CT], BF16)
    ar_out[(0, 0)] = nc.dram_tensor("ar_out0", [NCORES * P, CT], BF16,
                                    addr_space="Shared")
    for t in range(1, N_ITER + 1):
        for h in range(2):
            ar_in[(t, h)] = nc.dram_tensor(f"ar_in{t}_{h}", [P, CT], BF16)
            ar_out[(t, h)] = nc.dram_tensor(
                f"ar_out{t}_{h}", [NCORES * P, CT], BF16, addr_space="Shared")

    with tile.TileContext(nc) as tc:
        with tc.tile_pool(name="persist", bufs=1) as pp, \
             tc.tile_pool(name="kres", bufs=1) as pkr:
            ident8 = pp.tile([P, P], FP8, tag="ident8")
            make_identity(nc, ident8[:])
            m100 = pp.tile([P, 1], F32, tag="m100")
            nc.vector.memset(m100[:], -INV_REG)
            nbias = pp.tile([P, 1], F32, tag="nbias")
            nc.vector.memset(nbias[:], float(B))

            sd100 = pp.tile([P, RT], F32, tag="sd100")
            nc.sync.dma_start(out=sd100[:], in_=sd100_in[:])
            dsel_f = pp.tile([P, RT, CT], F32, tag="dself")
            nc.sync.dma_start(out=dsel_f[:], in_=dsel_in[:])
            dsel_bf = pp.tile([P, RT, CT], BF16, tag="dselbf")
            nc.vector.tensor_copy(dsel_bf[:], dsel_f[:])
            w5_f = pp.tile([P, CT], F32, tag="w5f")

            # A8 resident in both orientations
            krm8 = pkr.tile([P, RT, B], FP8, tag="krm8")     # [r%128, rb, c]
            # kt8 as 64 per-cb tiles: each written by ONE full-tile copy
            # (partial writes of one big tile raced with the pass LDW reads)
            kt8s = [pkr.tile([P, R], FP8, tag=f"kt8_{cb}", name=f"kt8_{cb}")
                    for cb in range(CT)]

            # per-pass vectors: ywh[t][h][p, cb_local, 2]:
            # col0=y_t, col1=w_{t-1}; split in AG halves. PE reads each half
            # tile, which is written by exactly one full-tile DVE copy.
            ywh = [[pp.tile([P, 32, 2], BF16, tag=f"ywh{t}_{h}",
                            name=f"ywh{t}_{h}") for h in range(2)]
                   for t in range(N_ITER + 1)]
            # row-side vectors of pass 5 (f32) for the loss
            rr5_f = pp.tile([P, RT, 2], F32, tag="rr5f")
            uv5_bf = pp.tile([P, RT, 2], BF16, tag="uv5bf")

            dummy_bf = pp.tile([P, 2], BF16, tag="dummy")
            nc.vector.memset(dummy_bf[:], 1e-3)
            kd_f = pp.tile([P, RT], F32, tag="kd_f")
            kd_bf = pp.tile([P, RT], BF16, tag="kd_bf")
            vt2seed = pp.tile([P, RT], F32, tag="vt2seed")

            # ---------------- build ----------------
            with tc.tile_pool(name="bfeat", bufs=1) as pf, \
                 tc.tile_pool(name="bps", bufs=3, space="PSUM") as pps, \
                 tc.tile_pool(name="bpt", bufs=2, space="PSUM") as bpt, \
                 tc.tile_pool(name="bsd", bufs=1, space="PSUM") as psd:
                imgT_sb = pf.tile([P, 2, R], BF16, tag="imgT")
                txtT_sb = pf.tile([P, 2, B], BF16, tag="txtT")
                nc.gpsimd.dma_start(out=imgT_sb[:], in_=imgT[:])
                tch = B // 8
                dmaq = [nc.sync, nc.scalar, nc.gpsimd]
                for k in range(8):
                    dmaq[k % 3].dma_start(
                        out=txtT_sb[:, :, k * tch:(k + 1) * tch],
                        in_=txtT[:, :, k * tch:(k + 1) * tch])

                # kd = exp(100*sd - 100); input sd100 = -100*sdiag
                nc.scalar.activation(out=kd_f[:], in_=sd100[:], func=Exp,
                                     scale=-1.0, bias=m100[:])
                nc.vector.tensor_copy(kd_bf[:], kd_f[:])
                if debug_taps:
                    nc.sync.dma_start(out=dbg["kd"][:], in_=kd_f[:])

                # Build, pipelined per 512-col chunk ch: GEMM (all 8 rb) ->
                # scaled exp -> seed-sweep pairs + transposes for the 4
                # c-blocks of the chunk. Everything converges a few us after
                # the last exp instead of serializing GEMM/seed/transposes.
                eacc = pf.tile([P, RT, NCH], F32, tag="eacc")
                psum_sd = psd.tile([P, CT], F32, tag="psum_sd")
                for ch in range(NCH):
                    for rb in range(RT):
                        psum_s = pps.tile([P, CH], F32, tag="psum_s")
                        for dh in range(2):
                            nc.tensor.matmul(
                                psum_s[:],
                                lhsT=imgT_sb[:, dh, rb * P:(rb + 1) * P],
                                rhs=txtT_sb[:, dh, ch * CH:(ch + 1) * CH],
                                start=(dh == 0), stop=(dh == 1),
                                skip_group_check=True)
                        nc.scalar.activation(
                            out=krm8[:, rb, ch * CH:(ch + 1) * CH],
                            in_=psum_s[:], func=Exp,
                            scale=INV_REG, bias=sd100[:, rb:rb + 1],
                            accum_out=eacc[:, rb, ch:ch + 1])
                    for cb in range(4 * ch, 4 * ch + 4):
                        # seed sweep: colsums_K partials = A8^T @ kd
                        for rb in range(RT):
                            nc.tensor.matmul(
                                psum_sd[:, cb:cb + 1],
                                lhsT=krm8[:, rb, cb * P:(cb + 1) * P],
                                rhs=kd_bf[:, rb:rb + 1],
                                start=(rb == 0), stop=(rb == RT - 1),
                                skip_group_check=True)
                        # kt8 = transpose(krm8), fp8 stride-2 psum out
                        psum_t = bpt.tile([P, RT, P, 2], FP8, tag="psum_t")
                        for rb in range(RT):
                            nc.tensor.transpose(
                                psum_t[:, rb, :, 0],
                                krm8[:, rb, cb * P:(cb + 1) * P],
                                ident8[:])
                        nc.vector.tensor_copy(kt8s[cb][:],
                                              psum_t[:, :, :, 0])
                # chain2 seed: vt2_1 = 1 / rowsums(A)  (exact f32)
                rowsA = pf.tile([P, RT], F32, tag="rowsA")
                nc.vector.tensor_reduce(rowsA[:], eacc[:],
                                        axis=mybir.AxisListType.X, op=ADD)
                nc.vector.reciprocal(vt2seed[:], rowsA[:])
                if debug_taps:
                    nc.sync.dma_start(out=dbg["vt2seed"][:], in_=vt2seed[:])
                stg0 = pf.tile([P, CT], BF16, tag="stg0")
                nc.scalar.copy(out=stg0[:], in_=psum_sd[:])
                nc.sync.dma_start(out=ar_in[(0, 0)][:], in_=stg0[:])
                nc.gpsimd.collective_compute(
                    "AllGather", BYPASS, replica_groups=RG,
                    ins=[ar_in[(0, 0)][:]], outs=[ar_out[(0, 0)][:]])


                # AG0 gather + reduce -> y_1 = 1/colsums_K ; w_0 = 1/B
                gath0 = pf.tile([P, NCORES, CT], BF16, tag="gath0")
                src0 = ar_out[(0, 0)][:].rearrange("(k p) w -> p k w", p=P)
                nc.sync.dma_start(out=gath0[:, 0:4, :], in_=src0[:, 0:4, :])
                nc.scalar.dma_start(out=gath0[:, 4:8, :], in_=src0[:, 4:8, :])
                gg0 = pf.tile([P, CT], F32, tag="gg0")
                nc.vector.tensor_add(gg0[:], gath0[:, 0, :], gath0[:, 1, :])
                for k in range(2, NCORES):
                    nc.vector.tensor_add(gg0[:], gg0[:], gath0[:, k, :])
                y1 = pf.tile([P, CT], F32, tag="y1")
                nc.vector.reciprocal(y1[:], gg0[:])
                y1stg = pf.tile([P, CT, 2], F32, tag="y1stg")
                nc.vector.memset(y1stg[:, :, 1], BVAL)
                nc.vector.tensor_copy(y1stg[:, :, 0], y1[:])
                for h in range(2):
                    nc.vector.tensor_copy(ywh[1][h][:],
                                          y1stg[:, 32 * h:32 * h + 32, :])
                if debug_taps:
                    nc.sync.dma_start(out=dbg["y1"][:], in_=y1[:])
                    nc.sync.dma_start(out=dbg["gg0"][:], in_=gg0[:])
                    nc.sync.dma_start(out=dbg["krm8"][:], in_=krm8[:])
                    for cb in range(CT):
                        nc.sync.dma_start(out=dbg["kt8"][:, cb, :],
                                          in_=kt8s[cb][:])

            # ---------------- sinkhorn passes ----------------
            with tc.tile_pool(name="sps", bufs=2, space="PSUM") as ppa, \
                 tc.tile_pool(name="spb", bufs=2, space="PSUM") as ppb, \
                 tc.tile_pool(name="sag", bufs=2) as pag, \
                 tc.tile_pool(name="ssm", bufs=2) as psm:
                for t in range(1, N_ITER + 1):
                    # stage A: rows. psum_a[p, rb, ch] += A8 @ [y_t, w_{t-1}]
                    psum_a = ppa.tile([P, RT, 2], F32, tag="psum_a")
                    for half in range(2):
                        for rb in range(RT):
                            for cb in range(32 * half, 32 * half + 32):
                                # ONE start/stop for the whole sweep: start
                                # clears has_written bank-wide, so per-group
                                # starts on interleaved groups drop earlier
                                # groups' partials.
                                nc.tensor.matmul(
                                    psum_a[:, rb, :],
                                    lhsT=kt8s[cb][:, rb * P:(rb + 1) * P],
                                    rhs=(dummy_bf[:] if (probe_rhs_const
                                     and t == 1) else
                                     ywh[t][cb // 32][:, cb % 32, :]),
                                    start=(half == 0 and rb == 0
                                           and cb == 0),
                                    stop=(half == 1 and rb == RT - 1
                                          and cb == CT - 1),
                                    skip_group_check=True)
                    # x_t = recip(col0); vt2_t = BVAL*recip(col1) (t>1)
                    if t == N_ITER:
                        rr = rr5_f
                        uv_bf = uv5_bf
                    else:
                        rr = psm.tile([P, RT, 2], F32, tag="rr")
                        uv_bf = psm.tile([P, RT, 2], BF16, tag="uv_bf")
                    nc.vector.reciprocal(rr[:], psum_a[:])
                    if t > 1:
                        nc.scalar.mul(out=rr[:, :, 1], in_=rr[:, :, 1],
                                      mul=BVAL)
                    else:
                        nc.scalar.copy(out=rr[:, :, 1], in_=vt2seed[:])
                    nc.vector.tensor_copy(uv_bf[:], rr[:])
                    if debug_taps:
                        nc.sync.dma_start(out=dbg[f"rr{t}"][:], in_=rr[:])

                    # stage B: cols, 2 AG halves overlapped
                    psum_b = ppb.tile([P, CT, 2], F32, tag="psum_b")
                    for half in range(2):
                        for cb in range(32 * half, 32 * half + 32):
                            for rb in range(RT):
                                nc.tensor.matmul(
                                    psum_b[:, cb, :],
                                    lhsT=krm8[:, rb, cb * P:(cb + 1) * P],
                                    rhs=uv_bf[:, rb, :],
                                    start=(rb == 0), stop=(rb == RT - 1),
                                    skip_group_check=True)
                        stg = pag.tile([P, CT], BF16, tag=f"stg{half}",
                                       name=f"stg{half}")
                        nc.vector.tensor_copy(
                            stg[:], psum_b[:, 32 * half:32 * half + 32, :])
                        nc.gpsimd.dma_start(out=ar_in[(t, half)][:],
                                            in_=stg[:])
                        nc.gpsimd.collective_compute(
                            "AllGather", BYPASS, replica_groups=RG,
                            ins=[ar_in[(t, half)][:]],
                            outs=[ar_out[(t, half)][:]])

                    # gather + reduce each half -> y_{t+1}, w_t
                    for half in range(2):
                        gath = pag.tile([P, NCORES, CT], BF16,
                                        tag=f"gath{half}", name=f"gath{half}")
                        src = ar_out[(t, half)][:].rearrange(
                            "(k p) w -> p k w", p=P)
                        nc.sync.dma_start(out=gath[:, 0:4, :],
                                          in_=src[:, 0:4, :])
                        nc.scalar.dma_start(out=gath[:, 4:8, :],
                                            in_=src[:, 4:8, :])
                        gg = psm.tile([P, CT], F32, tag=f"gg{half}",
                                      name=f"gg{half}")
                        nc.vector.tensor_add(gg[:], gath[:, 0, :],
                                             gath[:, 1, :])
                        for k in range(2, NCORES):
                            nc.vector.tensor_add(gg[:], gg[:], gath[:, k, :])
                        if debug_taps:
                            nc.sync.dma_start(out=dbg[f"gg{t}"][:, half, :],
                                              in_=gg[:])
                        rec = psm.tile([P, CT], F32, tag=f"rec{half}",
                                       name=f"rec{half}")
                        nc.vector.reciprocal(rec[:], gg[:])
                        rv = rec[:].rearrange("p (c v) -> p c v", v=2)
                        if t < N_ITER:
                            # y_{t+1} = BVAL*recip ; w_t = recip
                            ystg = psm.tile([P, 32, 2], F32,
                                            tag=f"ystg{half}",
                                            name=f"ystg{half}")
                            nc.scalar.mul(out=ystg[:, :, 0], in_=rv[:, :, 0],
                                          mul=BVAL)
                            nc.vector.tensor_copy(ystg[:, :, 1], rv[:, :, 1])
                            nc.vector.tensor_copy(ywh[t + 1][half][:],
                                                  ystg[:])
                        else:
                            # keep w_5 (f32) for the loss diag
                            nc.vector.tensor_copy(
                                w5_f[:, 32 * half:32 * half + 32],
                                rv[:, :, 1])

            # ---------------- loss (diagonal only) ----------------
            with tc.tile_pool(name="lsm", bufs=1) as pls:
                # y5 diag (bf16, consistent with pass-5 stage A rhs)
                y5d = pls.tile([P, RT], F32, tag="y5d")
                w5d = pls.tile([P, RT], F32, tag="w5d")
                scr = pls.tile([P, CT], F32, tag="selscr")
                y5all = pls.tile([P, CT], BF16, tag="y5all")
                for h in range(2):
                    nc.vector.tensor_copy(y5all[:, 32 * h:32 * h + 32],
                                          ywh[N_ITER][h][:, :, 0])
                for rb in range(RT):
                    nc.vector.scalar_tensor_tensor(
                        out=scr[:], in0=y5all[:], scalar=1.0,
                        in1=dsel_bf[:, rb, :], op0=MULT, op1=MULT,
                        accum_out=y5d[:, rb:rb + 1])
                    nc.vector.scalar_tensor_tensor(
                        out=scr[:], in0=w5_f[:], scalar=1.0,
                        in1=dsel_f[:, rb, :], op0=MULT, op1=MULT,
                        accum_out=w5d[:, rb:rb + 1])
                # p1 = x5 * y5d ; p2 = w5d * vt2_5bf   (A8_ii == 1 exactly)
                # per-chain ops so chain1 can run while the last AG is in
                # flight (chain2 needs w5_f from it)
                lsum = pls.tile([P, 2], F32, tag="lsum")
                pd1 = pls.tile([P, RT], F32, tag="pd1")
                pd2 = pls.tile([P, RT], F32, tag="pd2")
                nc.vector.tensor_mul(pd1[:], rr5_f[:, :, 0], y5d[:])
                v25f = pls.tile([P, RT], F32, tag="v25f")
                nc.vector.tensor_copy(v25f[:], uv5_bf[:, :, 1])
                nc.vector.tensor_mul(pd2[:], w5d[:], v25f[:])
                # loss rows: log(n + exp(p) - p) - p
                for ci, pd in enumerate((pd1, pd2)):
                    e = pls.tile([P, RT], F32, tag=f"e{ci}", name=f"e{ci}")
                    nc.scalar.activation(out=e[:], in_=pd[:], func=Exp)
                    nc.vector.tensor_sub(e[:], e[:], pd[:])
                    l_ = pls.tile([P, RT], F32, tag=f"l{ci}", name=f"l{ci}")
                    nc.scalar.activation(out=l_[:], in_=e[:], func=Log,
                                         bias=nbias[:])
                    nc.vector.tensor_sub(l_[:], l_[:], pd[:])
                    nc.vector.tensor_reduce(lsum[:, ci:ci + 1], l_[:],
                                            axis=mybir.AxisListType.X,
                                            op=ADD)
                nc.sync.dma_start(out=out_loss[:], in_=lsum[:])

    nc.compile()
    return nc


_NC_CACHE = None


def _get_nc():
    global _NC_CACHE
    if _NC_CACHE is None:
        _NC_CACHE = _build_bass()
    return _NC_CACHE


def make_in_maps(all_image_features, all_text_features):
    img = np.asarray(all_image_features, np.float32)
    txt = np.asarray(all_text_features, np.float32)

    img_bf = img.astype(ml_dtypes.bfloat16)
    txt_bf = txt.astype(ml_dtypes.bfloat16)
    # [d, x] -> [dlo, dhi, x] with d = dhi*128 + dlo
    imgT = np.ascontiguousarray(
        img_bf.T.reshape(2, P, B).transpose(1, 0, 2))
    txtT = np.ascontiguousarray(
        txt_bf.T.reshape(2, P, B).transpose(1, 0, 2))

    # sd100 = -100 * S_rr with S from the bf16 features (matches device GEMM
    # up to f32 reorder noise ~1e-6, far below the fp8 grid at 1.0)
    sdiag = np.einsum("bd,bd->b",
                      img_bf.astype(np.float32), txt_bf.astype(np.float32))
    sd100 = (-INV_REG * sdiag).astype(np.float32)

    in_maps = []
    for c in range(NCORES):
        rows = slice(c * R, (c + 1) * R)
        sd = np.ascontiguousarray(
            sd100[rows].reshape(RT, P).T).astype(np.float32)  # [p, rb]
        dsel = np.zeros((P, RT, CT), np.float32)
        for rb in range(RT):
            dsel[:, rb, c * RT + rb] = 1.0
        in_maps.append({
            "imgT": np.ascontiguousarray(imgT[:, :, rows]),
            "txtT": txtT,
            "sd100": sd,
            "dsel": dsel,
        })
    return in_maps


def kernel(all_image_features, all_text_features, logit_scale, labels):
    in_maps = make_in_maps(all_image_features, all_text_features)
    nc = _get_nc()
    res = bass_utils.run_bass_kernel_spmd(
        nc, in_maps, core_ids=list(range(NCORES)))
    tot = np.float64(0.0)
    for c in range(NCORES):
        tot += np.asarray(res.results[c]["out_loss"], np.float64).sum()
    return np.asarray(tot / (2 * B), dtype=np.float32)


# revision 4
# speedup vs baseline: 1.0385x; 1.0385x over previous
"""Entropic Sinkhorn loss kernel v2 for Trainium2 (8 NeuronCores, SPMD).

Math: loss = (sinkhorn(1 - img@txt.T) + sinkhorn((1 - img@txt.T).T)) / 2,
K = exp(100*S - 100), 5 iterations, P = u*K*v, loss = -mean(log_softmax(P)[i,i]).

Key ideas vs the baseline:
  * Row-scaled matrix A = diag(1/k_ii) K. Sinkhorn is exactly covariant under
    diagonal scaling: carrying x_t = kd*u_t, y_t = v_t (chain1) and
    w_t = u2_t, vt2_t = kd*v2_t (chain2) reproduces the reference exactly,
    with P recovered as diag products of the carried quantities (scales
    cancel). A has unit diagonal -> fits fp8 (off-diag underflows to 0 as it
    already does in bf16).
  * A stored fp8 in BOTH orientations (8MB + 8MB SBUF per core): kt8 (c-major,
    for row matvecs) and krm8 (r-major, for col matvecs). Every Sinkhorn
    stage is a dense 512-pair LDW+MM sweep on PE (mixed fp8 weights x bf16
    vectors, FWL) -- no transposes, copies or DVE matvecs in the pass loop.
  * Build: one GEMM (row-major S), ACT exp with per-partition bias
    (-100*S_rr) writes krm8 directly in fp8 and accumulates exact f32 A-row
    sums (chain2 seed). kt8 = PE transposes of krm8 (bitwise consistent).
    Chain1 seed from a seed sweep A8^T @ kd.
  * Stage-B AllGathers split in 2 halves and overlapped with compute; the
    next pass's stage A starts on the first half. Final loss reduced on
    device to [128, 2] per core and summed on host (no final collective).
"""

import numpy as np
import ml_dtypes

import concourse.bacc as bacc
import concourse.tile as tile
import concourse.mybir as mybir
from concourse import bass_utils
from concourse.masks import make_identity

B = 8192
D = 256
P = 128
NCORES = 8
R = B // NCORES            # 1024 rows per core
RT = R // P                # 8 row tiles per core
CT = B // P                # 64 col tiles
NCH = 16                   # GEMM free chunks per rb (8192 / 512)
CH = 512
N_ITER = 5
INV_REG = 100.0
BVAL = 1.0 / B

BF16 = mybir.dt.bfloat16
F32 = mybir.dt.float32
FP8 = mybir.dt.float8e4
Exp = mybir.ActivationFunctionType.Exp
Log = mybir.ActivationFunctionType.Ln
ADD = mybir.AluOpType.add
MULT = mybir.AluOpType.mult
BYPASS = mybir.AluOpType.bypass


def _build_bass(debug_taps=False, probe_rhs_const=False):
    nc = bacc.Bacc("TRN2", target_bir_lowering=False, debug=False,
                   num_devices=NCORES)
    dbg = {}
    if debug_taps:
        dbg["kd"] = nc.dram_tensor("dbg_kd", [P, RT], F32,
                                   kind="ExternalOutput")
        dbg["vt2seed"] = nc.dram_tensor("dbg_vt2seed", [P, RT], F32,
                                        kind="ExternalOutput")
        dbg["y1"] = nc.dram_tensor("dbg_y1", [P, CT], F32,
                                   kind="ExternalOutput")
        dbg["gg0"] = nc.dram_tensor("dbg_gg0", [P, CT], F32,
                                    kind="ExternalOutput")
        dbg["krm8"] = nc.dram_tensor("dbg_krm8", [P, RT, B], FP8,
                                     kind="ExternalOutput")
        dbg["kt8"] = nc.dram_tensor("dbg_kt8", [P, CT, R], FP8,
                                    kind="ExternalOutput")
        for t in range(1, N_ITER + 1):
            dbg[f"rr{t}"] = nc.dram_tensor(f"dbg_rr{t}", [P, RT, 2], F32,
                                           kind="ExternalOutput")
            dbg[f"gg{t}"] = nc.dram_tensor(f"dbg_gg{t}", [P, 2, CT], F32,
                                           kind="ExternalOutput")

    imgT = nc.dram_tensor("imgT", [P, 2, R], BF16, kind="ExternalInput")
    txtT = nc.dram_tensor("txtT", [P, 2, B], BF16, kind="ExternalInput")
    sd100_in = nc.dram_tensor("sd100", [P, RT], F32, kind="ExternalInput")
    dsel_in = nc.dram_tensor("dsel", [P, RT, CT], F32, kind="ExternalInput")
    out_loss = nc.dram_tensor("out_loss", [P, 2], F32, kind="ExternalOutput")

    RG = [list(range(NCORES))]
    # AG staging: t=0 seed [P, CT]; passes t=1..5: 2 halves of [P, CT] each
    ar_in = {}
    ar_out = {}
    ar_in[(0, 0)] = nc.dram_tensor("ar_in0", [P, CT], BF16)
    ar_out[(0, 0)] = nc.dram_tensor("ar_out0", [NCORES * P, CT], BF16,
                                    addr_space="Shared")
    for t in range(1, N_ITER + 1):
        for h in range(2):
            ar_in[(t, h)] = nc.dram_tensor(f"ar_in{t}_{h}", [P, CT], BF16)
            ar_out[(t, h)] = nc.dram_tensor(
                f"ar_out{t}_{h}", [NCORES * P, CT], BF16, addr_space="Shared")

    with tile.TileContext(nc) as tc:
        with tc.tile_pool(name="persist", bufs=1) as pp, \
             tc.tile_pool(name="kres", bufs=1) as pkr:
            ident8 = pp.tile([P, P], FP8, tag="ident8")
            make_identity(nc, ident8[:])
            m100 = pp.tile([P, 1], F32, tag="m100")
            nc.vector.memset(m100[:], -INV_REG)
            nbias = pp.tile([P, 1], F32, tag="nbias")
            nc.vector.memset(nbias[:], float(B))

            sd100 = pp.tile([P, RT], F32, tag="sd100")
            nc.sync.dma_start(out=sd100[:], in_=sd100_in[:])
            dsel_f = pp.tile([P, RT, CT], F32, tag="dself")
            nc.sync.dma_start(out=dsel_f[:], in_=dsel_in[:])
            dsel_bf = pp.tile([P, RT, CT], BF16, tag="dselbf")
            nc.vector.tensor_copy(dsel_bf[:], dsel_f[:])
            w5_f = pp.tile([P, CT], F32, tag="w5f")

            # A8 resident in both orientations
            krm8 = pkr.tile([P, RT, B], FP8, tag="krm8")     # [r%128, rb, c]
            # kt8 as 64 per-cb tiles: each written by ONE full-tile copy
            # (partial writes of one big tile raced with the pass LDW reads)
            kt8s = [pkr.tile([P, R], FP8, tag=f"kt8_{cb}", name=f"kt8_{cb}")
                    for cb in range(CT)]

            # per-pass vectors: ywh[t][h][p, cb_local, 2]:
            # col0=y_t, col1=w_{t-1}; split in AG halves. PE reads each half
            # tile, which is written by exactly one full-tile DVE copy.
            ywh = [[pp.tile([P, 32, 2], BF16, tag=f"ywh{t}_{h}",
                            name=f"ywh{t}_{h}") for h in range(2)]
                   for t in range(N_ITER + 1)]
            # row-side vectors of pass 5 (f32) for the loss
            rr5_f = pp.tile([P, RT, 2], F32, tag="rr5f")
            uv5_bf = pp.tile([P, RT, 2], BF16, tag="uv5bf")

            dummy_bf = pp.tile([P, 2], BF16, tag="dummy")
            nc.vector.memset(dummy_bf[:], 1e-3)
            kd_f = pp.tile([P, RT], F32, tag="kd_f")
            kd_bf = pp.tile([P, RT], BF16, tag="kd_bf")
            vt2seed = pp.tile([P, RT], F32, tag="vt2seed")

            # ---------------- build ----------------
            with tc.tile_pool(name="bfeat", bufs=1) as pf, \
                 tc.tile_pool(name="bps", bufs=3, space="PSUM") as pps, \
                 tc.tile_pool(name="bpt", bufs=2, space="PSUM") as bpt, \
                 tc.tile_pool(name="bsd", bufs=1, space="PSUM") as psd:
                imgT_sb = pf.tile([P, 2, R], BF16, tag="imgT")
                txtT_sb = pf.tile([P, 2, B], BF16, tag="txtT")
                nc.gpsimd.dma_start(out=imgT_sb[:], in_=imgT[:])
                tch = B // 8
                dmaq = [nc.sync, nc.scalar, nc.gpsimd]
                for k in range(8):
                    dmaq[k % 3].dma_start(
                        out=txtT_sb[:, :, k * tch:(k + 1) * tch],
                        in_=txtT[:, :, k * tch:(k + 1) * tch])

                # kd = exp(100*sd - 100); input sd100 = -100*sdiag
                nc.scalar.activation(out=kd_f[:], in_=sd100[:], func=Exp,
                                     scale=-1.0, bias=m100[:])
                nc.vector.tensor_copy(kd_bf[:], kd_f[:])
                if debug_taps:
                    nc.sync.dma_start(out=dbg["kd"][:], in_=kd_f[:])

                # Build, pipelined per 512-col chunk ch: GEMM (all 8 rb) ->
                # scaled exp -> seed-sweep pairs + transposes for the 4
                # c-blocks of the chunk. Everything converges a few us after
                # the last exp instead of serializing GEMM/seed/transposes.
                eacc = pf.tile([P, RT, NCH], F32, tag="eacc")
                psum_sd = psd.tile([P, CT], F32, tag="psum_sd")
                for ch in range(NCH):
                    for rb in range(RT):
                        psum_s = pps.tile([P, CH], F32, tag="psum_s")
                        for dh in range(2):
                            nc.tensor.matmul(
                                psum_s[:],
                                lhsT=imgT_sb[:, dh, rb * P:(rb + 1) * P],
                                rhs=txtT_sb[:, dh, ch * CH:(ch + 1) * CH],
                                start=(dh == 0), stop=(dh == 1),
                                skip_group_check=True)
                        nc.scalar.activation(
                            out=krm8[:, rb, ch * CH:(ch + 1) * CH],
                            in_=psum_s[:], func=Exp,
                            scale=INV_REG, bias=sd100[:, rb:rb + 1],
                            accum_out=eacc[:, rb, ch:ch + 1])
                    for cb in range(4 * ch, 4 * ch + 4):
                        # seed sweep: colsums_K partials = A8^T @ kd
                        for rb in range(RT):
                            nc.tensor.matmul(
                                psum_sd[:, cb:cb + 1],
                                lhsT=krm8[:, rb, cb * P:(cb + 1) * P],
                                rhs=kd_bf[:, rb:rb + 1],
                                start=(rb == 0), stop=(rb == RT - 1),
                                skip_group_check=True)
                        # kt8 = transpose(krm8), fp8 stride-2 psum out
                        psum_t = bpt.tile([P, RT, P, 2], FP8, tag="psum_t")
                        for rb in range(RT):
                            nc.tensor.transpose(
                                psum_t[:, rb, :, 0],
                                krm8[:, rb, cb * P:(cb + 1) * P],
                                ident8[:])
                        nc.vector.tensor_copy(kt8s[cb][:],
                                              psum_t[:, :, :, 0])
                # chain2 seed: vt2_1 = 1 / rowsums(A)  (exact f32)
                rowsA = pf.tile([P, RT], F32, tag="rowsA")
                nc.vector.tensor_reduce(rowsA[:], eacc[:],
                                        axis=mybir.AxisListType.X, op=ADD)
                nc.vector.reciprocal(vt2seed[:], rowsA[:])
                if debug_taps:
                    nc.sync.dma_start(out=dbg["vt2seed"][:], in_=vt2seed[:])
                stg0 = pf.tile([P, CT], BF16, tag="stg0")
                nc.scalar.copy(out=stg0[:], in_=psum_sd[:])
                nc.sync.dma_start(out=ar_in[(0, 0)][:], in_=stg0[:])
                nc.gpsimd.collective_compute(
                    "AllGather", BYPASS, replica_groups=RG,
                    ins=[ar_in[(0, 0)][:]], outs=[ar_out[(0, 0)][:]])


                # AG0 gather + reduce -> y_1 = 1/colsums_K ; w_0 = 1/B
                gath0 = pf.tile([P, NCORES, CT], BF16, tag="gath0")
                src0 = ar_out[(0, 0)][:].rearrange("(k p) w -> p k w", p=P)
                nc.sync.dma_start(out=gath0[:, 0:4, :], in_=src0[:, 0:4, :])
                nc.scalar.dma_start(out=gath0[:, 4:8, :], in_=src0[:, 4:8, :])
                gg0 = pf.tile([P, CT], F32, tag="gg0")
                nc.vector.tensor_add(gg0[:], gath0[:, 0, :], gath0[:, 1, :])
                for k in range(2, NCORES):
                    nc.vector.tensor_add(gg0[:], gg0[:], gath0[:, k, :])
                y1 = pf.tile([P, CT], F32, tag="y1")
                nc.vector.reciprocal(y1[:], gg0[:])
                y1stg = pf.tile([P, CT, 2], F32, tag="y1stg")
                nc.vector.memset(y1stg[:, :, 1], BVAL)
                nc.vector.tensor_copy(y1stg[:, :, 0], y1[:])
                for h in range(2):
                    nc.vector.tensor_copy(ywh[1][h][:],
                                          y1stg[:, 32 * h:32 * h + 32, :])
                if debug_taps:
                    nc.sync.dma_start(out=dbg["y1"][:], in_=y1[:])
                    nc.sync.dma_start(out=dbg["gg0"][:], in_=gg0[:])
                    nc.sync.dma_start(out=dbg["krm8"][:], in_=krm8[:])
                    for cb in range(CT):
                        nc.sync.dma_start(out=dbg["kt8"][:, cb, :],
                                          in_=kt8s[cb][:])

            # ---------------- sinkhorn passes ----------------
            with tc.tile_pool(name="sps", bufs=2, space="PSUM") as ppa, \
                 tc.tile_pool(name="spb", bufs=2, space="PSUM") as ppb, \
                 tc.tile_pool(name="sag", bufs=2) as pag, \
                 tc.tile_pool(name="ssm", bufs=2) as psm:
                for t in range(1, N_ITER + 1):
                    # stage A: rows. psum_a[p, rb, ch] += A8 @ [y_t, w_{t-1}]
                    psum_a = ppa.tile([P, RT, 2], F32, tag="psum_a")
                    for half in range(2):
                        for rb in range(RT):
                            for cb in range(32 * half, 32 * half + 32):
                                # ONE start/stop for the whole sweep: start
                                # clears has_written bank-wide, so per-group
                                # starts on interleaved groups drop earlier
                                # groups' partials.
                                nc.tensor.matmul(
                                    psum_a[:, rb, :],
                                    lhsT=kt8s[cb][:, rb * P:(rb + 1) * P],
                                    rhs=(dummy_bf[:] if (probe_rhs_const
                                     and t == 1) else
                                     ywh[t][cb // 32][:, cb % 32, :]),
                                    start=(half == 0 and rb == 0
                                           and cb == 0),
                                    stop=(half == 1 and rb == RT - 1
                                          and cb == CT - 1),
                                    skip_group_check=True)
                    # x_t = recip(col0); vt2_t = BVAL*recip(col1) (t>1)
                    # BVAL gauges dropped: per-iterate scalar factors
                    # cancel exactly in the diag products (b = 2^-13 is a
                    # power of two -> bf16 roundings unchanged).
                    if t == N_ITER:
                        uv_bf = uv5_bf
                        nc.vector.reciprocal(rr5_f[:], psum_a[:])
                        nc.vector.tensor_copy(uv_bf[:], rr5_f[:])
                    else:
                        uv_bf = psm.tile([P, RT, 2], BF16, tag="uv_bf")
                        with nc.allow_low_precision(
                                reason="gauge-invariant iterate"):
                            nc.vector.reciprocal(uv_bf[:], psum_a[:])
                    if t == 1:
                        nc.vector.tensor_copy(uv_bf[:, :, 1], vt2seed[:])
                    if debug_taps:
                        nc.sync.dma_start(out=dbg[f"rr{t}"][:], in_=rr[:])

                    # stage B: cols, 2 AG halves overlapped
                    psum_b = ppb.tile([P, CT, 2], F32, tag="psum_b")
                    for half in range(2):
                        for cb in range(32 * half, 32 * half + 32):
                            for rb in range(RT):
                                nc.tensor.matmul(
                                    psum_b[:, cb, :],
                                    lhsT=krm8[:, rb, cb * P:(cb + 1) * P],
                                    rhs=uv_bf[:, rb, :],
                                    start=(rb == 0), stop=(rb == RT - 1),
                                    skip_group_check=True)
                        stg = pag.tile([P, CT], BF16, tag=f"stg{half}",
                                       name=f"stg{half}")
                        nc.vector.tensor_copy(
                            stg[:], psum_b[:, 32 * half:32 * half + 32, :])
                        nc.gpsimd.dma_start(out=ar_in[(t, half)][:],
                                            in_=stg[:])
                        nc.gpsimd.collective_compute(
                            "AllGather", BYPASS, replica_groups=RG,
                            ins=[ar_in[(t, half)][:]],
                            outs=[ar_out[(t, half)][:]])

                    # gather + reduce each half -> y_{t+1}, w_t
                    for half in range(2):
                        gath = pag.tile([P, NCORES, CT], BF16,
                                        tag=f"gath{half}", name=f"gath{half}")
                        src = ar_out[(t, half)][:].rearrange(
                            "(k p) w -> p k w", p=P)
                        nc.sync.dma_start(out=gath[:, 0:4, :],
                                          in_=src[:, 0:4, :])
                        nc.scalar.dma_start(out=gath[:, 4:8, :],
                                            in_=src[:, 4:8, :])
                        gg = psm.tile([P, CT], F32, tag=f"gg{half}",
                                      name=f"gg{half}")
                        nc.vector.tensor_add(gg[:], gath[:, 0, :],
                                             gath[:, 1, :])
                        for k in range(2, NCORES):
                            nc.vector.tensor_add(gg[:], gg[:], gath[:, k, :])
                        if debug_taps:
                            nc.sync.dma_start(out=dbg[f"gg{t}"][:, half, :],
                                              in_=gg[:])
                        if t < N_ITER:
                            with nc.allow_low_precision(
                                    reason="gauge-invariant iterate"):
                                nc.vector.reciprocal(
                                    ywh[t + 1][half][:],
                                    gg[:].rearrange("p (c v) -> p c v", v=2))
                        else:
                            rec = psm.tile([P, CT], F32, tag=f"rec{half}",
                                           name=f"rec{half}")
                            nc.vector.reciprocal(rec[:], gg[:])
                            rv = rec[:].rearrange("p (c v) -> p c v", v=2)
                            nc.vector.tensor_copy(
                                w5_f[:, 32 * half:32 * half + 32],
                                rv[:, :, 1])

            # ---------------- loss (diagonal only) ----------------
            with tc.tile_pool(name="lsm", bufs=1) as pls:
                # y5 diag (bf16, consistent with pass-5 stage A rhs)
                y5d = pls.tile([P, RT], F32, tag="y5d")
                w5d = pls.tile([P, RT], F32, tag="w5d")
                scr = pls.tile([P, CT], F32, tag="selscr")
                y5all = pls.tile([P, CT], BF16, tag="y5all")
                for h in range(2):
                    nc.vector.tensor_copy(y5all[:, 32 * h:32 * h + 32],
                                          ywh[N_ITER][h][:, :, 0])
                for rb in range(RT):
                    nc.vector.scalar_tensor_tensor(
                        out=scr[:], in0=y5all[:], scalar=1.0,
                        in1=dsel_bf[:, rb, :], op0=MULT, op1=MULT,
                        accum_out=y5d[:, rb:rb + 1])
                    nc.vector.scalar_tensor_tensor(
                        out=scr[:], in0=w5_f[:], scalar=1.0,
                        in1=dsel_f[:, rb, :], op0=MULT, op1=MULT,
                        accum_out=w5d[:, rb:rb + 1])
                # p1 = x5 * y5d ; p2 = w5d * vt2_5bf   (A8_ii == 1 exactly)
                # per-chain ops so chain1 can run while the last AG is in
                # flight (chain2 needs w5_f from it)
                lsum = pls.tile([P, 2], F32, tag="lsum")
                pd1 = pls.tile([P, RT], F32, tag="pd1")
                pd2 = pls.tile([P, RT], F32, tag="pd2")
                nc.vector.tensor_mul(pd1[:], rr5_f[:, :, 0], y5d[:])
                v25f = pls.tile([P, RT], F32, tag="v25f")
                nc.vector.tensor_copy(v25f[:], uv5_bf[:, :, 1])
                nc.vector.tensor_mul(pd2[:], w5d[:], v25f[:])
                # loss rows: log(n + exp(p) - p) - p
                for ci, pd in enumerate((pd1, pd2)):
                    e = pls.tile([P, RT], F32, tag=f"e{ci}", name=f"e{ci}")
                    nc.scalar.activation(out=e[:], in_=pd[:], func=Exp)
                    nc.vector.tensor_sub(e[:], e[:], pd[:])
                    l_ = pls.tile([P, RT], F32, tag=f"l{ci}", name=f"l{ci}")
                    nc.scalar.activation(out=l_[:], in_=e[:], func=Log,
                                         bias=nbias[:])
                    nc.vector.tensor_sub(l_[:], l_[:], pd[:])
                    nc.vector.tensor_reduce(lsum[:, ci:ci + 1], l_[:],
                                            axis=mybir.AxisListType.X,
                                            op=ADD)
                nc.sync.dma_start(out=out_loss[:], in_=lsum[:])

    nc.compile()
    return nc


_NC_CACHE = None


def _get_nc():
    global _NC_CACHE
    if _NC_CACHE is None:
        _NC_CACHE = _build_bass()
    return _NC_CACHE


def make_in_maps(all_image_features, all_text_features):
    img = np.asarray(all_image_features, np.float32)
    txt = np.asarray(all_text_features, np.float32)

    img_bf = img.astype(ml_dtypes.bfloat16)
    txt_bf = txt.astype(ml_dtypes.bfloat16)
    # [d, x] -> [dlo, dhi, x] with d = dhi*128 + dlo
    imgT = np.ascontiguousarray(
        img_bf.T.reshape(2, P, B).transpose(1, 0, 2))
    txtT = np.ascontiguousarray(
        txt_bf.T.reshape(2, P, B).transpose(1, 0, 2))

    # sd100 = -100 * S_rr with S from the bf16 features (matches device GEMM
    # up to f32 reorder noise ~1e-6, far below the fp8 grid at 1.0)
    sdiag = np.einsum("bd,bd->b",
                      img_bf.astype(np.float32), txt_bf.astype(np.float32))
    sd100 = (-INV_REG * sdiag).astype(np.float32)

    in_maps = []
    for c in range(NCORES):
        rows = slice(c * R, (c + 1) * R)
        sd = np.ascontiguousarray(
            sd100[rows].reshape(RT, P).T).astype(np.float32)  # [p, rb]
        dsel = np.zeros((P, RT, CT), np.float32)
        for rb in range(RT):
            dsel[:, rb, c * RT + rb] = 1.0
        in_maps.append({
            "imgT": np.ascontiguousarray(imgT[:, :, rows]),
            "txtT": txtT,
            "sd100": sd,
            "dsel": dsel,
        })
    return in_maps


def kernel(all_image_features, all_text_features, logit_scale, labels):
    in_maps = make_in_maps(all_image_features, all_text_features)
    nc = _get_nc()
    res = bass_utils.run_bass_kernel_spmd(
        nc, in_maps, core_ids=list(range(NCORES)))
    tot = np.float64(0.0)
    for c in range(NCORES):
        tot += np.asarray(res.results[c]["out_loss"], np.float64).sum()
    return np.asarray(tot / (2 * B), dtype=np.float32)
